# revision 47
# baseline (speedup 1.0000x reference)
"""Trainium2 Bass kernel for nn_BodyFaceEmotionClassifier.

Pipeline (per reference):
  concat(body, hand_r, hand_l) -> [B,T,67,3]; gate (x,y) by conf>0.1 ->
  pos [B,T,134]; relu(pos@W1+b1); masked max pool over valid t;
  BatchNorm over batch; classifier @Wc+bc -> [64, 7].

Strategy (8 NeuronCores, pure data parallel over batch):
  * Host specializes on the runtime `length` values: batches sorted by
    length, dealt into 8 slots x 8 cores; slot j has one compile-time
    length L_j (group max rounded to 128) so a single SPMD program fits
    every core.  Short batches are padded by repeating their own first
    row (duplicates never change a max-pool).
  * All streamed data ships fp16 (halves HBM traffic, 1 PE cycle/row
    matmuls, ~8e-4 pipeline rel err).  Conf values only feed the >0.1
    predicate, so the host rounds them to fp16 NUDGED to preserve the
    fp32 predicate exactly under either scalar-compare semantics; the
    device still evaluates the compare+mult gate itself.
  * maint [128, 2V] interleaves per-chunk [coords | conf-dup] blocks so
    ONE Sync-queue DMA brings a whole 1024-col chunk (the second HWDGE
    queue carries consts; GpSimd SWDGE carries the rem stream --
    spreading issue cost over three queues; the DMA-issue sequencer is
    a real serial resource).
  * Gate: one fused DVE op per chunk [128, n]: (conf > 0.1) * coord,
    fp16 out.  The 6 leftover features (x64..66, y64..66) are host
    pre-gated [6, V]; on device they live in a ring of [128, CHUNK]
    tiles whose rows 6:128 are zeroed ONCE -- the K=6 rem matmul is
    padded to K=128 because any K<128 matmul reconfigures the PE array
    and poisons the whole stream to ~2.5x cost (measured), while a
    full-K matmul with zero rows runs at full rate (~230-290 ns at
    N=512 fp16).
  * Chunking is a uniform 1024-col stream DECOUPLED from slot
    boundaries (no pipeline hiccups at short slots); pooling reduces
    run per (chunk x slot) segment.  Per chunk: 2 D-halves x 2 K-splits
    x 2 subs of N=512 matmuls accumulate into [128, 1024] 2-bank PSUM
    tiles; the Scalar engine drains PSUM to fp16 SBUF (frees banks
    early, offloads DVE; fp16 rounding is monotone so it commutes with
    the max-pool); DVE segment-reduces the fp16 copy into percol, and
    folds a slot's columns into pooled when its last segment lands.
  * bias+relu after pooling (commutes with max).  A tiny warm-up
    AllReduce, data-gated on chunk 0's load, pays the one-time CC mesh
    setup (~35us) in the shadow of the main loop.  BN batch stats come
    from a 2KB AllReduce(add) of per-core (sum x, sum x^2) over the 8
    pooled columns; each core then normalizes and classifies only its
    OWN 8 batches ([8, 7] out) -- no pooled gather, no transpose DMA,
    minimal post-collective chain.  The host reassembles [64, 7] from
    all 8 cores' outputs, undoing the sort permutation.

Measured on trn2 (8 cores): ~116-121us vs 289us baseline (~2.4x), rel
err 1.65e-3 (gate 2e-2).  PE/DVE/ACT each ~35-55% busy; remaining time
is inter-engine semaphore latency, ~12us startup (6us framework), the
collective peer wait (+-10us run variance), and ~10us teardown.
"""

import sys

for _p in ("/opt/trn_rl_repo", "/opt/trn_rl_repo/concourse"):
    if _p not in sys.path:
        sys.path.insert(0, _p)

import ml_dtypes
import numpy as np

NP16 = np.float16

import concourse.bacc as bacc
import concourse.mybir as mybir
import concourse.tile as tile
from concourse import bass_utils

# bass_utils imports antenv.axon_hooks when BASS_TRACE is set under axon;
# some images lack the module (profiling then degrades gracefully to a
# None hook instead of crashing on ImportError).
try:
    import antenv.axon_hooks  # noqa: F401
except ImportError:
    try:
        import types

        import antenv

        _m = types.ModuleType("antenv.axon_hooks")
        _m._AXON_NTFF_PROFILE_HOOK = None

        def _set_hook(h, _m=_m):
            _m._AXON_NTFF_PROFILE_HOOK = h

        def _get_hook(_m=_m):
            return _m._AXON_NTFF_PROFILE_HOOK

        _m.set_axon_ntff_profile_hook = _set_hook
        _m.get_axon_ntff_profile_hook = _get_hook
        sys.modules["antenv.axon_hooks"] = _m
        antenv.axon_hooks = _m
    except Exception:
        pass

F32 = mybir.dt.float32
F16 = mybir.dt.float16
BF16 = mybir.dt.bfloat16
AX = mybir.AxisListType
OP = mybir.AluOpType
ACT = mybir.ActivationFunctionType

B, T = 64, 4096
K = 67          # keypoints
NF = 134        # 2K gated coord features
NRAW = 201      # 3K raw features
D = 256
C = 7
THR = 0.1
EPS = 1e-5
NCORES = 8
P = 128
KM = 128        # main contraction rows (x0..63, y0..63)
RK = 6          # remainder contraction rows (x64..66, y64..66)
CHUNK = 1024
SUB = 512


def _plan(lengths):
    """Sort batches desc, deal into 8 slots x 8 cores, pad slot length to
    the group max rounded up to a multiple of 128."""
    order = np.argsort(-lengths, kind="stable")
    L = []
    assign = np.empty((NCORES, NCORES), dtype=np.int64)  # [core, slot] -> batch
    for j in range(NCORES):
        grp = order[NCORES * j : NCORES * (j + 1)]
        L.append(int(-(-int(lengths[grp].max()) // P) * P))
        for c in range(NCORES):
            assign[c, j] = grp[c]
    return L, assign


def _chunks(Lj):
    off = 0
    while off < Lj:
        n = min(CHUNK, Lj - off)
        yield off, n
        off += n


def _subs(n):
    off = 0
    while off < n:
        s = min(SUB, n - off)
        yield off, s
        off += s


def _nchunks(L):
    return sum(1 for Lj in L for _ in _chunks(Lj))


def _stream(L):
    """Uniform CHUNK-sized tiles over the whole packed stream, decoupled
    from slot boundaries (so the compute pipeline never hiccups at short
    slot tails).  Yields (roff, n, segs) with segs = [(a, b, j, done)]:
    half-open column ranges [a, b) within the chunk belonging to slot j;
    done marks the segment that finishes slot j."""
    V = sum(L)
    bounds = []
    s = 0
    for Lj in L:
        bounds.append((s, s + Lj))
        s += Lj
    roff = 0
    while roff < V:
        n = min(CHUNK, V - roff)
        segs = []
        for j, (s0, s1) in enumerate(bounds):
            a = max(s0, roff)
            b = min(s1, roff + n)
            if a < b:
                segs.append((a - roff, b - roff, j, b == s1))
        yield roff, n, segs
        roff += n


def _build(L, stop_after="full"):
    """Build + compile the SPMD Bass program for slot lengths L."""
    V = sum(L)
    nseg = sum(len(segs) for _, _, segs in _stream(L))

    nc = bacc.Bacc(
        "TRN2", target_bir_lowering=False, debug=False, num_devices=NCORES
    )

    maint_d = nc.dram_tensor("maint", [P, 2 * V], F16, kind="ExternalInput")
    remg_d = nc.dram_tensor("remg", [RK, V], F16, kind="ExternalInput")
    w1a_d = nc.dram_tensor("w1a", [KM, D], F16, kind="ExternalInput")
    w1b_d = nc.dram_tensor("w1b", [P, D], F16, kind="ExternalInput")
    b1_d = nc.dram_tensor("b1", [D, 1], F32, kind="ExternalInput")
    gamma_d = nc.dram_tensor("gamma", [D, 1], F32, kind="ExternalInput")
    beta_d = nc.dram_tensor("beta", [D, 1], F32, kind="ExternalInput")
    wc_d = nc.dram_tensor("wc", [D, C], F32, kind="ExternalInput")
    bc_d = nc.dram_tensor("bc", [B, C], F32, kind="ExternalInput")
    out_d = nc.dram_tensor("out", [NCORES, C], F32, kind="ExternalOutput")

    with tile.TileContext(nc) as tc:
        with (
            tc.tile_pool(name="consts", bufs=1) as consts,
            tc.tile_pool(name="dram", bufs=1, space="DRAM") as dram,
            tc.tile_pool(name="apool", bufs=7) as apool,
            tc.tile_pool(name="gpool", bufs=4) as gpool,
            tc.tile_pool(name="hpool", bufs=4) as hpool,
            tc.tile_pool(name="psS", bufs=2, space="PSUM") as psS,
        ):
            # consts load on the Activation HWDGE queue, keeping the Sync
            # queue free for the chunk stream
            w1a = consts.tile([KM, D], F16)
            nc.scalar.dma_start(w1a[:], w1a_d[:, :])
            w1b = consts.tile([P, D], F16)
            nc.scalar.dma_start(w1b[:], w1b_d[:, :])
            # rem moving-operand ring: K padded 6 -> 128 with persistent
            # zero rows (a K<128 matmul reconfigures the PE array and
            # serializes the stream at ~2.5x cost; a full-K matmul with
            # zero rows runs at full rate).  DMA fills rows 0:6 per chunk;
            # rows 6:128 stay zero forever.
            NRG = 6
            rgz = [
                consts.tile([P, CHUNK], F16, name=f"rgz{i}")
                for i in range(NRG)
            ]
            for t in rgz:
                nc.gpsimd.memset(t[:], 0.0)
            b1h = consts.tile([P, 2], F32)
            nc.scalar.dma_start(b1h[:, 0:1], b1_d[0:P, :])
            nc.scalar.dma_start(b1h[:, 1:2], b1_d[P:D, :])
            gamh = consts.tile([P, 2], F32)
            nc.scalar.dma_start(gamh[:, 0:1], gamma_d[0:P, :])
            nc.scalar.dma_start(gamh[:, 1:2], gamma_d[P:D, :])
            beth = consts.tile([P, 2], F32)
            nc.scalar.dma_start(beth[:, 0:1], beta_d[0:P, :])
            nc.scalar.dma_start(beth[:, 1:2], beta_d[P:D, :])
            wch = consts.tile([P, 2 * C], F32)
            nc.scalar.dma_start(wch[:, 0:C], wc_d[0:P, :])
            nc.scalar.dma_start(wch[:, C : 2 * C], wc_d[P:D, :])
            bc_sb = consts.tile([B, C], F32)
            nc.scalar.dma_start(bc_sb[:], bc_d[:, :])

            percol = [
                consts.tile([P, nseg], F32, name=f"percol{h}")
                for h in range(2)
            ]
            pooled = [
                consts.tile([P, NCORES], F32, name=f"pooled{h}")
                for h in range(2)
            ]

            ci = 0
            cidx = 0
            slot_c0 = [None] * len(L)
            warmed = False
            for roff, n, segs in _stream(L):
                # ONE interleaved [128, 2n] load per chunk (coords then
                # conf) on the Sync queue; pre-gated rem [6, n] on the
                # GpSimd SWDGE queue into the zero-padded ring
                mt = apool.tile([P, 2 * CHUNK], F16, name="mt", tag="mt")
                rg = rgz[cidx % NRG]
                cidx += 1
                if cidx == 1:
                    # stripe chunk 0 at sub granularity (sub-0 coords,
                    # sub-0 conf first) so the first gate and matmuls
                    # start as early as possible
                    s0 = min(SUB, n)
                    nc.sync.dma_start(mt[:, 0:s0], maint_d[:, 0:s0])
                    nc.sync.dma_start(
                        mt[:, n : n + s0], maint_d[:, n : n + s0]
                    )
                    if n > s0:
                        nc.sync.dma_start(
                            mt[:, s0:n], maint_d[:, s0:n]
                        )
                        nc.sync.dma_start(
                            mt[:, n + s0 : 2 * n],
                            maint_d[:, n + s0 : 2 * n],
                        )
                else:
                    nc.sync.dma_start(
                        mt[:, 0 : 2 * n],
                        maint_d[:, 2 * roff : 2 * roff + 2 * n],
                    )
                nc.gpsimd.dma_start(
                    out=rg[0:RK, 0:n], in_=remg_d[:, roff : roff + n]
                )
                if not warmed and stop_after in ("collective", "full"):
                    # warm-up AllGather: pays the one-time CC mesh setup
                    # in the shadow of the main loop.  Gated on chunk 0's
                    # load (via the wu_in copy) so the CC rings don't
                    # compete with the pipeline ramp-up.
                    warmed = True
                    wu_in = dram.tile([1, 8], F16)
                    wu_out = dram.tile([1, 8], F16)
                    nc.sync.dma_start(wu_in[:, :], mt[0:1, 0:8])
                    nc.gpsimd.collective_compute(
                        "AllReduce",
                        OP.add,
                        replica_groups=[list(range(NCORES))],
                        ins=[wu_in[:].opt()],
                        outs=[wu_out[:].opt()],
                    )
                # main gate: one dense fused op [128, n] (chunk 0 gates
                # per sub to overlap its striped load); conf compares
                # against the fp32 scalar, gated coords written fp16
                pt = gpool.tile([P, CHUNK], F16, name="pt", tag="pt")
                if cidx == 1:
                    for so, sn in _subs(n):
                        nc.vector.scalar_tensor_tensor(
                            out=pt[:, so : so + sn],
                            in0=mt[:, n + so : n + so + sn],
                            scalar=THR,
                            in1=mt[:, so : so + sn],
                            op0=OP.is_gt,
                            op1=OP.mult,
                        )
                else:
                    nc.vector.scalar_tensor_tensor(
                        out=pt[:, 0:n],
                        in0=mt[:, n : 2 * n],
                        scalar=THR,
                        in1=mt[:, 0:n],
                        op0=OP.is_gt,
                        op1=OP.mult,
                    )
                sf = [
                    psS.tile([P, CHUNK], F32, name=f"sf{h}", tag=f"s{h}")
                    for h in range(2)
                ]
                for so, sn in _subs(n):
                    for h in range(2):
                        nc.tensor.matmul(
                            sf[h][:, so : so + sn],
                            w1a[:, h * P : (h + 1) * P],
                            pt[:, so : so + sn],
                            start=True,
                            stop=False,
                        )
                        nc.tensor.matmul(
                            sf[h][:, so : so + sn],
                            w1b[:, h * P : (h + 1) * P],
                            rg[:, so : so + sn],
                            start=False,
                            stop=True,
                        )
                sfh = [None, None]
                for h in range(2):
                    # drain PSUM to fp16 SBUF on the (otherwise idle)
                    # Scalar engine: frees the PSUM banks early and halves
                    # the DVE read cost; monotone rounding commutes with
                    # the max-pool
                    sfh[h] = hpool.tile(
                        [P, CHUNK], F16, name=f"sfh{h}", tag=f"sfh{h}"
                    )
                    nc.scalar.copy(sfh[h][:, 0:n], sf[h][:, 0:n])
                for a, b, j, done in segs:
                    if slot_c0[j] is None:
                        slot_c0[j] = ci
                    for h in range(2):
                        nc.vector.tensor_reduce(
                            percol[h][:, ci : ci + 1],
                            sfh[h][:, a:b],
                            axis=AX.X,
                            op=OP.max,
                        )
                    ci += 1
                    if done:
                        # slot complete: fold its percol columns
                        for h in range(2):
                            nc.vector.tensor_reduce(
                                pooled[h][:, j : j + 1],
                                percol[h][:, slot_c0[j] : ci],
                                axis=AX.X,
                                op=OP.max,
                            )
            assert ci == nseg
            if stop_after == "mainloop":
                nc.sync.dma_start(out_d[:, :], pooled[0][0:B, 0:C])

            # bias + relu (commute with max-pool)
            prelu = [
                consts.tile([P, NCORES], F32, name=f"prelu{h}")
                for h in range(2)
            ]
            for h in range(2):
                nc.scalar.activation(
                    prelu[h][:],
                    pooled[h][:],
                    ACT.Relu,
                    bias=b1h[:, h : h + 1],
                    scale=1.0,
                )
            if stop_after == "prelu":
                nc.sync.dma_start(out_d[:, :], prelu[0][0:B, 0:C])

            # BN stats via AllReduce of per-core (sum x, sum x^2):
            # each core then normalizes and classifies only ITS OWN 8
            # batches ([8, 7] output) -- no pooled gather, no transpose
            # DMA, tiny post-collective chain.  Host reassembles [64, 7]
            # from all 8 cores' outputs.
            if stop_after == "full":
                part = consts.tile([P, 4], F32)
                sqs = [
                    consts.tile([P, NCORES], F32, name=f"sq{h}")
                    for h in range(2)
                ]
                for h in range(2):
                    nc.vector.tensor_reduce(
                        part[:, 2 * h : 2 * h + 1],
                        prelu[h][:],
                        axis=AX.X,
                        op=OP.add,
                    )
                    nc.scalar.activation(
                        sqs[h][:],
                        prelu[h][:],
                        ACT.Square,
                        accum_out=part[:, 2 * h + 1 : 2 * h + 2],
                    )
                red_in = dram.tile([P, 4], F32)
                red_out = dram.tile([P, 4], F32)
                nc.sync.dma_start(red_in[:, :], part[:])
                nc.gpsimd.collective_compute(
                    "AllReduce",
                    OP.add,
                    replica_groups=[list(range(NCORES))],
                    ins=[red_in[:].opt()],
                    outs=[red_out[:].opt()],
                )
                stat = consts.tile([P, 4], F32)
                nc.sync.dma_start(stat[:], red_out[:, :])
                epsc = consts.tile([P, 1], F32)
                nc.vector.memset(epsc[:], EPS)
                scr = consts.tile([P, 16], F32)
                bnT = [
                    consts.tile([P, NCORES], F32, name=f"bnT{h}")
                    for h in range(2)
                ]
                for h in range(2):
                    o = 8 * h
                    mean = scr[:, o + 0 : o + 1]
                    esq = scr[:, o + 1 : o + 2]
                    msq = scr[:, o + 2 : o + 3]
                    var = scr[:, o + 3 : o + 4]
                    sd = scr[:, o + 4 : o + 5]
                    rstd = scr[:, o + 5 : o + 6]
                    scl = scr[:, o + 6 : o + 7]
                    shift = scr[:, o + 7 : o + 8]
                    nc.vector.tensor_scalar_mul(
                        mean, stat[:, 2 * h : 2 * h + 1], 1.0 / B
                    )
                    nc.vector.tensor_mul(msq, mean, mean)
                    # var = E[x^2] - mean^2 = ssq/B - msq
                    nc.vector.scalar_tensor_tensor(
                        out=var,
                        in0=stat[:, 2 * h + 1 : 2 * h + 2],
                        scalar=1.0 / B,
                        in1=msq,
                        op0=OP.mult,
                        op1=OP.subtract,
                    )
                    nc.scalar.activation(sd, var, ACT.Sqrt, bias=epsc[:])
                    nc.vector.reciprocal(rstd, sd)
                    nc.vector.tensor_mul(scl, gamh[:, h : h + 1], rstd)
                    ms = scr[:, o + 1 : o + 2]  # esq dead now: reuse
                    nc.vector.tensor_mul(ms, mean, scl)
                    nc.vector.tensor_sub(shift, beth[:, h : h + 1], ms)
                    nc.scalar.activation(
                        bnT[h][:], prelu[h][:], ACT.Identity,
                        bias=shift, scale=scl,
                    )
                out_ps = psS.tile([NCORES, C], F32, name="ops", tag="s0")
                nc.tensor.matmul(
                    out_ps[:], bnT[0][:], wch[:, 0:C], start=True, stop=False
                )
                nc.tensor.matmul(
                    out_ps[:], bnT[1][:], wch[:, C : 2 * C],
                    start=False, stop=True,
                )
                osb = consts.tile([NCORES, C], F32)
                nc.vector.tensor_add(osb[:], out_ps[:], bc_sb[0:NCORES, :])
                nc.sync.dma_start(out_d[:, :], osb[:])

    nc.compile()
    return nc, V


_CACHE = {}


def _get_program(L):
    key = tuple(L)
    if key not in _CACHE:
        _CACHE[key] = _build(list(L))
    return _CACHE[key]


def _nudge_conf16(cf32):
    """fp16-round conf so the device's (conf > 0.1) predicate matches the
    fp32 reference exactly, whether the scalar compares as fp32(0.1) or
    fp16(0.1): above-threshold values are forced strictly above fp32(0.1)
    and the rest to at most fp16(0.1) (the smaller of the two)."""
    want = cf32 > np.float32(THR)
    ch = cf32.astype(NP16)
    chf = ch.astype(np.float32)
    lo = NP16(THR)                      # 0.0999755859375 <= both thresholds
    hi = np.nextafter(lo, NP16(np.inf))  # 0.10003662109375 > fp32(0.1)
    ch = np.where(want & ~(chf > np.float32(THR)), hi, ch)
    ch = np.where(~want & (chf > lo.astype(np.float32)), lo, ch)
    return ch


def _pack_inputs(body, hand_right, hand_left, lengths, L, assign, V):
    """Per-core inputs (all fp16): maint [128, 2V] with per-chunk
    interleaving -- for each 1024-col chunk at row-offset r, columns
    2r:2r+n hold the coords (rows x0..63,y0..63) and columns 2r+n:2r+2n
    hold the conf rows (c0..63 twice, nudged to preserve the >0.1
    predicate) -- so one DMA brings a whole chunk.  remg [6, V] is
    feature-major pre-gated (x64..66, y64..66 times (conf>0.1)).
    Padding rows repeat the batch's first row."""
    maint_all, remg_all = [], []
    chunk_bounds = [(roff, n) for roff, n, _ in _stream(L)]
    assert sum(n for _, n in chunk_bounds) == V
    for c in range(NCORES):
        buf = np.empty((V, NRAW), dtype=np.float32)
        off = 0
        for j, Lj in enumerate(L):
            b = int(assign[c, j])
            lb = int(lengths[b])
            row = np.concatenate(
                (body[b, :lb], hand_right[b, :lb], hand_left[b, :lb]), axis=1
            )
            buf[off : off + lb] = row
            if Lj > lb:
                buf[off + lb : off + Lj] = row[0]
            off += Lj
        assert off == V
        coords = np.empty((P, V), dtype=NP16)
        coords[0:64] = buf[:, 0 : 3 * 64 : 3].T.astype(NP16)   # x0..63
        coords[64:128] = buf[:, 1 : 3 * 64 : 3].T.astype(NP16)  # y0..63
        confs = np.empty((P, V), dtype=NP16)
        confs[0:64] = _nudge_conf16(buf[:, 2 : 3 * 64 : 3].T)     # c0..63
        confs[64:128] = confs[0:64]
        maint = np.empty((P, 2 * V), dtype=NP16)
        for r, n in chunk_bounds:
            maint[:, 2 * r : 2 * r + n] = coords[:, r : r + n]
            maint[:, 2 * r + n : 2 * r + 2 * n] = confs[:, r : r + n]
        g = (buf[:, 194:201:3] > THR).astype(np.float32)  # c64..66 gate
        remg = np.empty((RK, V), dtype=NP16)
        remg[0:3] = (buf[:, 192:201:3] * g).T.astype(NP16)  # x64..66
        remg[3:6] = (buf[:, 193:201:3] * g).T.astype(NP16)  # y64..66
        maint_all.append(np.ascontiguousarray(maint))
        remg_all.append(np.ascontiguousarray(remg))
    return maint_all, remg_all


def _make_base(W1, b1, gamma, beta, Wc, bc):
    W1 = np.asarray(W1, dtype=np.float32)
    # w1a row order matches maint rows: x0..63 -> W1[2k], y0..63 -> W1[2k+1]
    w1a = np.concatenate((W1[0 : 2 * 64 : 2], W1[1 : 2 * 64 : 2]), axis=0)
    # w1b row order matches remg rows: x64..66 -> W1[2k], y64..66 -> W1[2k+1];
    # zero-padded to K=128 (rows 6:128) to keep the PE pipeline full-rate
    w1b = np.zeros((P, D), dtype=np.float32)
    w1b[0:3] = W1[2 * 64 :: 2]
    w1b[3:6] = W1[2 * 64 + 1 :: 2]
    return {
        "w1a": np.ascontiguousarray(w1a.astype(NP16)),
        "w1b": np.ascontiguousarray(w1b.astype(NP16)),
        "b1": np.asarray(b1, np.float32).reshape(D, 1).copy(),
        "gamma": np.asarray(gamma, np.float32).reshape(D, 1).copy(),
        "beta": np.asarray(beta, np.float32).reshape(D, 1).copy(),
        "wc": np.ascontiguousarray(np.asarray(Wc, np.float32)),
        "bc": np.broadcast_to(
            np.asarray(bc, np.float32).reshape(1, C), (B, C)
        ).copy(),
    }


def kernel(body, hand_right, hand_left, length, W1, b1, gamma, beta, Wc, bc):
    lengths = np.asarray(length).astype(np.int64)
    body = np.asarray(body, dtype=np.float32)
    hand_right = np.asarray(hand_right, dtype=np.float32)
    hand_left = np.asarray(hand_left, dtype=np.float32)

    L, assign = _plan(lengths)
    nc, V = _get_program(L)
    maint_all, remg_all = _pack_inputs(
        body, hand_right, hand_left, lengths, L, assign, V
    )
    base = _make_base(W1, b1, gamma, beta, Wc, bc)
    in_maps = [
        dict(base, maint=maint_all[c], remg=remg_all[c])
        for c in range(NCORES)
    ]

    res = bass_utils.run_bass_kernel_spmd(
        nc, in_maps, core_ids=list(range(NCORES))
    )
    kernel.last_results = res

    out = np.empty((B, C), dtype=np.float32)
    for c in range(NCORES):
        oc = res.results[c]["out"]  # core c: row s = batch assign[c, s]
        for s in range(NCORES):
            out[int(assign[c, s])] = oc[s]
    return out
